# revision 1
# baseline (speedup 1.0000x reference)
"""Trainium2 Bass kernel for ExplainableDumplingGNN (MPNN -> 3x GAT -> SAGE -> pool).

Self-contained: takes full inputs, shards node blocks + incident edges across
8 NeuronCores internally, runs one SPMD Bass kernel, returns [64, 2] log-probs.

Sharding: core c owns nodes [1250c, 1250c+1250), padded to 1280 rows so every
core has exactly 10 dst blocks of 128. Edges are assigned to the core owning
their dst, sorted by dst, grouped per 128-node dst block, padded to a uniform
number of 128-edge tiles. Per-edge gathers use the gpsimd dma_gather ucode
(<=1024 indices per instruction; int16 indices replicated across the 8 Q7
cores' partition groups). Segment softmax uses exp(alpha) with no max
subtraction (alpha stays in [-12, 9] for this input family); the weighted
scatter-add is a one-hot matmul accumulating in PSUM per dst block. Features
and matmuls are bf16 with fp32 accumulation.
"""
import sys

sys.path.insert(0, "/opt/trn_rl_repo")

import ml_dtypes
import numpy as np

import concourse.bacc as bacc
import concourse.bass as bass
import concourse.mybir as mybir
import concourse.tile as tile
from concourse import bass_utils
from concourse.masks import make_identity

P = 128
NCORES = 8
N = 10000
NBLK = 1250
NPAD = 1280
BLOCKS = 10
NFULL = NPAD * NCORES  # 10240
D_IN = 8
HID = 64
HEADS = 8
HC = 512
G = 64
XG = 64  # padded x row for MPNN dma_gather (256B rows)
MAXT = 8  # max 128-edge tiles per dma_gather (1024 descriptors)

F32 = mybir.dt.float32
BF = mybir.dt.bfloat16
I16 = mybir.dt.int16

BF_NP = ml_dtypes.bfloat16

_CACHE = {}


def _chunks(K):
    out = []
    k0 = 0
    while k0 < K:
        n = min(MAXT, K - k0)
        out.append((k0, n))
        k0 += n
    return out


def _pad_id(n):
    return (n // NBLK) * NPAD + (n % NBLK)


def _split_blocks(es_pad, ed_local):
    order = np.argsort(ed_local, kind="stable")
    es_pad, ed_local = es_pad[order], ed_local[order]
    per_block = []
    K = 1
    for b in range(BLOCKS):
        m = (ed_local >= b * P) & (ed_local < (b + 1) * P)
        s, d = es_pad[m], ed_local[m] - b * P
        per_block.append((s, d))
        K = max(K, (len(s) + P - 1) // P)
    return per_block, K


def _pack_idx16(flat):
    """[n] int -> [128, n//16] int16, wrapped in 16 partitions, replicated x8."""
    n = len(flat)
    ncols = n // 16
    a = np.zeros((P, ncols), np.int16)
    j = np.arange(n)
    a[j % 16, j // 16] = flat.astype(np.int16)
    for c in range(1, 8):
        a[16 * c:16 * (c + 1)] = a[:16]
    return a


def _finalize_edge_arrays(per_block, K, dt_np):
    """Returns (src_flat [BLOCKS, K*P], dstl [P, T], mask [P, T])."""
    T = BLOCKS * K
    src_flat = np.zeros((BLOCKS, K * P), np.int32)
    dstl = np.zeros((P, T), dt_np)
    mask = np.zeros((P, T), dt_np)
    for b, (s, d) in enumerate(per_block):
        n = len(s)
        slots = K * P
        s_pad = np.zeros(slots, np.int32)
        d_pad = np.zeros(slots, np.int32)
        m_pad = np.zeros(slots, np.float32)
        s_pad[:n] = s
        d_pad[:n] = d
        m_pad[:n] = 1.0
        if 0 < n < slots:
            s_pad[n:] = s[n - 1]
            d_pad[n:] = d[n - 1]
        src_flat[b] = s_pad
        for k in range(K):
            t = b * K + k
            sl = slice(k * P, (k + 1) * P)
            dstl[:, t] = d_pad[sl].astype(dt_np)
            mask[:, t] = m_pad[sl].astype(dt_np)
    return src_flat, dstl, mask


def _pack_block_idx(src_flat, K):
    """src_flat [BLOCKS, K*P] -> packed int16 [128, BLOCKS * K*P//16]."""
    cols = K * P // 16
    out = np.zeros((P, BLOCKS * cols), np.int16)
    for b in range(BLOCKS):
        out[:, b * cols:(b + 1) * cols] = _pack_idx16(src_flat[b])
    return out


def _preprocess(inputs):
    x = np.asarray(inputs["x"], np.float32)
    ei = np.asarray(inputs["edge_index"], np.int32)
    batch = np.asarray(inputs["batch"], np.int32)
    src, dst = ei[0], ei[1]

    blocks_per_core = []
    K_gat = 1
    K_sage = 1
    for c in range(NCORES):
        lo, hi = c * NBLK, (c + 1) * NBLK
        m = (dst >= lo) & (dst < hi)
        s_c = _pad_id(src[m]).astype(np.int32)
        d_c = (dst[m] - lo).astype(np.int32)
        own = np.arange(lo, hi, dtype=np.int32)
        gs = np.concatenate([s_c, _pad_id(own).astype(np.int32)])
        gd = np.concatenate([d_c, (own - lo)])
        gat_blocks, kg = _split_blocks(gs, gd)
        sage_blocks, ks = _split_blocks(s_c, d_c)
        K_gat = max(K_gat, kg)
        K_sage = max(K_sage, ks)
        blocks_per_core.append((gat_blocks, sage_blocks))

    per_core = []
    for c in range(NCORES):
        gat_blocks, sage_blocks = blocks_per_core[c]
        gsrc_flat, gdstl, gmask = _finalize_edge_arrays(gat_blocks, K_gat, BF_NP)
        ssrc_flat, sdstl, smask = _finalize_edge_arrays(sage_blocks, K_sage,
                                                        np.float32)
        # xr row index per edge: b*128 + dstl  (within [0, NPAD))
        gxr_flat = np.zeros_like(gsrc_flat)
        for b in range(BLOCKS):
            dl = gdstl[:, b * K_gat:(b + 1) * K_gat].astype(np.float32)
            # rebuild flat order (tile-major)
            gxr_flat[b] = (b * P + dl.T.ravel()).astype(np.int32)
        per_core.append(dict(
            gat_idx16=_pack_block_idx(gsrc_flat, K_gat),
            gxr_idx16=_pack_block_idx(gxr_flat, K_gat),
            sage_idx16=_pack_block_idx(ssrc_flat, K_sage),
            gdstl=gdstl, gmask=gmask, sdstl=sdstl, smask=smask,
        ))

    B_all = []
    for c in range(NCORES):
        Bm = np.zeros((P, BLOCKS * G), np.float32)
        loc = np.arange(NBLK)
        gids = batch[c * NBLK:(c + 1) * NBLK]
        Bm[loc % P, (loc // P) * G + gids] = 1.0
        B_all.append(Bm.astype(BF_NP))

    gcnt = np.bincount(batch, minlength=G).astype(np.float32)
    recip_gcnt = (1.0 / np.maximum(gcnt, 1.0)).reshape(G, 1).astype(np.float32)

    x_gather = np.zeros((NFULL, XG), np.float32)
    for c in range(NCORES):
        x_gather[c * NPAD:c * NPAD + NBLK, :D_IN] = x[c * NBLK:(c + 1) * NBLK]
    x_gather[:, D_IN] = 1.0
    xT_aug = []
    for c in range(NCORES):
        t = np.zeros((D_IN + 1, NPAD), np.float32)
        t[:D_IN, :NBLK] = x[c * NBLK:(c + 1) * NBLK].T
        t[D_IN, :] = 1.0
        xT_aug.append(t)

    w = {}
    w["mlw_aug"] = np.concatenate(
        [np.asarray(inputs["mpnn_lin_w"], np.float32),
         np.asarray(inputs["mpnn_lin_b"], np.float32)[None, :]], axis=0)
    w["muw"] = np.asarray(inputs["mpnn_upd_w"], np.float32)
    w["mub_rep"] = np.tile(np.asarray(inputs["mpnn_upd_b"], np.float32)[None, :], (P, 1))
    for i in (1, 2, 3):
        w[f"wl{i}"] = np.asarray(inputs[f"g{i}_wl"], np.float32).astype(BF_NP)
        w[f"wr{i}"] = np.asarray(inputs[f"g{i}_wr"], np.float32).astype(BF_NP)
        w[f"wres{i}"] = np.asarray(inputs[f"g{i}_res"], np.float32).astype(BF_NP)
        w[f"att_rep{i}"] = np.tile(
            np.asarray(inputs[f"g{i}_att"], np.float32).reshape(1, HC),
            (P, 1)).astype(BF_NP)
        w[f"b_rep{i}"] = np.tile(
            np.asarray(inputs[f"g{i}_b"], np.float32)[None, :], (P, 1))
    w["sage_wn"] = np.asarray(inputs["sage_wn"], np.float32).astype(BF_NP)
    w["sage_wr"] = np.asarray(inputs["sage_wr"], np.float32).astype(BF_NP)
    w["sbn_rep"] = np.tile(np.asarray(inputs["sage_bn"], np.float32)[None, :], (P, 1))
    w["out_w"] = np.asarray(inputs["out_w"], np.float32)
    w["ob_rep"] = np.tile(np.asarray(inputs["out_b"], np.float32)[None, :], (G, 1))

    return dict(
        K_gat=K_gat, K_sage=K_sage, per_core=per_core,
        B_all=B_all, recip_gcnt=recip_gcnt,
        x_gather=x_gather, xT_aug=xT_aug, weights=w,
    )


def _build(K_gat, K_sage):
    nc = bacc.Bacc("TRN2", target_bir_lowering=False, debug=False,
                   num_devices=NCORES)

    TG = BLOCKS * K_gat
    TS = BLOCKS * K_sage
    GCOLS = K_gat * P // 16   # idx16 cols per block (GAT)
    SCOLS = K_sage * P // 16

    x_gather_in = nc.dram_tensor("x_gather", [NFULL, XG], F32, kind="ExternalInput")
    xT_aug = nc.dram_tensor("xT_aug", [D_IN + 1, NPAD], F32, kind="ExternalInput")
    gat_idx_in = nc.dram_tensor("gat_idx16", [P, BLOCKS * GCOLS], I16,
                                kind="ExternalInput")
    gxr_idx_in = nc.dram_tensor("gxr_idx16", [P, BLOCKS * GCOLS], I16,
                                kind="ExternalInput")
    sage_idx_in = nc.dram_tensor("sage_idx16", [P, BLOCKS * SCOLS], I16,
                                 kind="ExternalInput")
    gat_dstl = nc.dram_tensor("gat_dstl", [P, TG], BF, kind="ExternalInput")
    gat_mask = nc.dram_tensor("gat_mask", [P, TG], BF, kind="ExternalInput")
    sage_dstl = nc.dram_tensor("sage_dstl", [P, TS], F32, kind="ExternalInput")
    sage_mask = nc.dram_tensor("sage_mask", [P, TS], F32, kind="ExternalInput")
    B_in = nc.dram_tensor("B_onehot", [P, BLOCKS * G], BF, kind="ExternalInput")
    rgc_in = nc.dram_tensor("recip_gcnt", [G, 1], F32, kind="ExternalInput")

    mlw_aug_in = nc.dram_tensor("mlw_aug", [D_IN + 1, HID], F32, kind="ExternalInput")
    muw_in = nc.dram_tensor("muw", [2 * HID, HID], F32, kind="ExternalInput")
    mub_in = nc.dram_tensor("mub_rep", [P, HID], F32, kind="ExternalInput")
    wls, wrs, wress, atts, brs = {}, {}, {}, {}, {}
    for i in (1, 2, 3):
        ind = HID if i == 1 else HC
        wls[i] = nc.dram_tensor(f"wl{i}", [ind, HC], BF, kind="ExternalInput")
        wrs[i] = nc.dram_tensor(f"wr{i}", [ind, HC], BF, kind="ExternalInput")
        wress[i] = nc.dram_tensor(f"wres{i}", [ind, HC], BF, kind="ExternalInput")
        atts[i] = nc.dram_tensor(f"att_rep{i}", [P, HC], BF, kind="ExternalInput")
        brs[i] = nc.dram_tensor(f"b_rep{i}", [P, HC], F32, kind="ExternalInput")
    swn_in = nc.dram_tensor("sage_wn", [HC, HID], BF, kind="ExternalInput")
    swr_in = nc.dram_tensor("sage_wr", [HC, HID], BF, kind="ExternalInput")
    sbn_in = nc.dram_tensor("sbn_rep", [P, HID], F32, kind="ExternalInput")
    ow_in = nc.dram_tensor("out_w", [HID, 2], F32, kind="ExternalInput")
    ob_in = nc.dram_tensor("ob_rep", [G, 2], F32, kind="ExternalInput")

    out = nc.dram_tensor("out", [G, 2], F32, kind="ExternalOutput")

    gat_ch = _chunks(K_gat)
    sage_ch = _chunks(K_sage)

    with tile.TileContext(nc) as tc:
        with (
            tc.tile_pool(name="const", bufs=1) as cp,
            tc.tile_pool(name="hTp", bufs=2) as hTp,
            tc.tile_pool(name="dram", bufs=1, space="DRAM") as dr,
        ):
            ident = cp.tile([P, P], F32)
            make_identity(nc, ident[:])
            ident_bf = cp.tile([P, P], BF)
            nc.vector.tensor_copy(ident_bf[:], ident[:])
            iota_i = cp.tile([P, P], mybir.dt.int32)
            nc.gpsimd.iota(iota_i[:], pattern=[[1, P]], base=0, channel_multiplier=0)
            iota_f = cp.tile([P, P], F32)
            nc.vector.tensor_copy(iota_f[:], iota_i[:])
            iota_bf = cp.tile([P, P], BF)
            nc.vector.tensor_copy(iota_bf[:], iota_i[:])
            ones_col = cp.tile([P, 1], BF)
            nc.gpsimd.memset(ones_col[:], 1.0)

            gat_idx = cp.tile([P, BLOCKS * GCOLS], I16)
            nc.sync.dma_start(gat_idx[:], gat_idx_in[:])
            gxr_idx = cp.tile([P, BLOCKS * GCOLS], I16)
            nc.sync.dma_start(gxr_idx[:], gxr_idx_in[:])
            sage_idx = cp.tile([P, BLOCKS * SCOLS], I16)
            nc.sync.dma_start(sage_idx[:], sage_idx_in[:])
            gdstl = cp.tile([P, TG], BF)
            nc.sync.dma_start(gdstl[:], gat_dstl[:])
            gmask = cp.tile([P, TG], BF)
            nc.sync.dma_start(gmask[:], gat_mask[:])
            sdstl = cp.tile([P, TS], F32)
            nc.sync.dma_start(sdstl[:], sage_dstl[:])
            smask = cp.tile([P, TS], F32)
            nc.sync.dma_start(smask[:], sage_mask[:])
            B_sb = cp.tile([P, BLOCKS * G], BF)
            nc.sync.dma_start(B_sb[:], B_in[:])
            rgc = cp.tile([G, 1], F32)
            nc.sync.dma_start(rgc[:], rgc_in[:])

            def gather_block(dst_blk, src_dram, idx_sb, b, chunks, cols, width):
                """Fill dst_blk [P, K*width] with gathered rows for block b."""
                for (k0, nt) in chunks:
                    nidx = nt * P
                    nc.gpsimd.dma_gather(
                        dst_blk[:, k0 * width:(k0 + nt) * width]
                            .rearrange("p (k d) -> p k d", k=nt),
                        src_dram[:],
                        idx_sb[:, b * cols + k0 * P // 16:
                               b * cols + (k0 + nt) * P // 16],
                        nidx, nidx, width)

            # =========================================================
            # Stage 0: MPNN (fp32) -> h1_own bf16 -> AllGather h1_full
            # =========================================================
            h1_bounce = dr.tile([NPAD, HID], BF)
            h1_full_d = dr.tile([NFULL, HID], BF, addr_space="Shared")

            with (
                tc.tile_pool(name="mp_sb", bufs=1) as wp,
                tc.tile_pool(name="mp_ps", bufs=1, space="PSUM") as pp,
            ):
                xT_sb = wp.tile([D_IN + 1, NPAD], F32)
                nc.sync.dma_start(xT_sb[:], xT_aug[:])
                mlw_sb = wp.tile([D_IN + 1, HID], F32)
                nc.sync.dma_start(mlw_sb[:], mlw_aug_in[:])
                muw_sb = wp.tile([2 * HID, HID], F32)
                nc.sync.dma_start(muw_sb[:], muw_in[:])
                mub_sb = wp.tile([P, HID], F32)
                nc.sync.dma_start(mub_sb[:], mub_in[:])

                for b in range(BLOCKS):
                    xgm_blk = wp.tile([P, K_sage * XG], F32, tag="xgm", bufs=2)
                    gather_block(xgm_blk, x_gather_in, sage_idx, b, sage_ch,
                                 SCOLS, XG)
                    xs_ps = pp.tile([P, D_IN + 1], F32, tag="xs", bufs=2, space="PSUM")
                    selm_blk = wp.tile([P, K_sage * P], F32, tag="selm", bufs=2)
                    nc.vector.tensor_tensor(
                        out=selm_blk[:].rearrange("p (k q) -> p k q", k=K_sage),
                        in0=sdstl[:, b * K_sage:(b + 1) * K_sage].unsqueeze(2)
                            .to_broadcast([P, K_sage, P]),
                        in1=iota_f[:].unsqueeze(1).to_broadcast([P, K_sage, P]),
                        op=mybir.AluOpType.is_equal)
                    nc.vector.tensor_mul(
                        selm_blk[:].rearrange("p (k q) -> p k q", k=K_sage),
                        selm_blk[:].rearrange("p (k q) -> p k q", k=K_sage),
                        smask[:, b * K_sage:(b + 1) * K_sage].unsqueeze(2)
                            .to_broadcast([P, K_sage, P]))
                    for k in range(K_sage):
                        nc.tensor.matmul(
                            xs_ps[:], lhsT=selm_blk[:, k * P:(k + 1) * P],
                            rhs=xgm_blk[:, k * XG:k * XG + D_IN + 1],
                            start=(k == 0), stop=(k == K_sage - 1))
                    xs_sb = wp.tile([P, D_IN + 1], F32, tag="xs_sb", bufs=2)
                    nc.vector.tensor_copy(xs_sb[:], xs_ps[:])
                    xsT_ps = pp.tile([D_IN + 1, P], F32, tag="tr", bufs=2, space="PSUM")
                    nc.tensor.transpose(xsT_ps[:], xs_sb[:], ident[:])
                    xsT_sb = wp.tile([D_IN + 1, P], F32, tag="xsT_sb", bufs=2)
                    nc.vector.tensor_copy(xsT_sb[:], xsT_ps[:])

                    m_ps = pp.tile([P, HID], F32, tag="m", bufs=1, space="PSUM")
                    nc.tensor.matmul(m_ps[:], lhsT=xsT_sb[:], rhs=mlw_sb[:],
                                     start=True, stop=True)
                    xw_ps = pp.tile([P, HID], F32, tag="xw", bufs=1, space="PSUM")
                    nc.tensor.matmul(xw_ps[:], lhsT=xT_sb[:, b * P:(b + 1) * P],
                                     rhs=mlw_sb[:], start=True, stop=True)
                    xw_sb = wp.tile([P, HID], F32, tag="xw_sb", bufs=2)
                    nc.vector.tensor_copy(xw_sb[:], xw_ps[:])
                    m_sb = wp.tile([P, HID], F32, tag="m_sb", bufs=2)
                    nc.vector.tensor_copy(m_sb[:], m_ps[:])
                    zcat = wp.tile([P, P], F32, tag="zcat", bufs=2)
                    zT_ps = pp.tile([HID, P], F32, tag="tr", bufs=2, space="PSUM")
                    nc.tensor.transpose(zT_ps[:], xw_sb[:], ident[:])
                    nc.vector.tensor_copy(zcat[:HID, :], zT_ps[:])
                    zT2_ps = pp.tile([HID, P], F32, tag="tr", bufs=2, space="PSUM")
                    nc.tensor.transpose(zT2_ps[:], m_sb[:], ident[:])
                    nc.vector.tensor_copy(zcat[HID:, :], zT2_ps[:])
                    h1_ps = pp.tile([P, HID], F32, tag="h1", bufs=1, space="PSUM")
                    nc.tensor.matmul(h1_ps[:], lhsT=zcat[:], rhs=muw_sb[:],
                                     start=True, stop=True)
                    h1_sb = wp.tile([P, HID], BF, tag="h1_sb", bufs=2)
                    nc.vector.tensor_add(h1_sb[:], h1_ps[:], mub_sb[:])
                    nc.scalar.activation(h1_sb[:], h1_sb[:],
                                         mybir.ActivationFunctionType.Relu)
                    nc.sync.dma_start(h1_bounce[b * P:(b + 1) * P, :], h1_sb[:])

            nc.gpsimd.collective_compute(
                "AllGather", mybir.AluOpType.bypass,
                replica_groups=[list(range(NCORES))],
                ins=[h1_bounce.opt()], outs=[h1_full_d.opt()])

            NT = NFULL // P  # 80

            # =========================================================
            # GAT edge phase (bf16)
            # =========================================================
            def gat_edge_phase(layer, xl_dram, xr_dram, res_dram, hT_next,
                               h3_bounce=None):
                K = K_gat
                with (
                    tc.tile_pool(name=f"edge_sb{layer}", bufs=1) as wp,
                    tc.tile_pool(name=f"edge_ps{layer}", bufs=1,
                                 space="PSUM") as pp,
                ):
                    att_sb = wp.tile([P, HC], BF, tag="att")
                    nc.sync.dma_start(att_sb[:], atts[layer][:])
                    bias_sb = wp.tile([P, HC], F32, tag="bias")
                    nc.sync.dma_start(bias_sb[:], brs[layer][:])
                    for b in range(BLOCKS):
                        out_ps = pp.tile([P, HC], F32, tag="outps", bufs=2,
                                         space="PSUM")
                        den_ps = pp.tile([P, HEADS], F32, tag="denps", bufs=2,
                                         space="PSUM")
                        xg_blk = wp.tile([P, K * HC], BF, tag="xg_blk", bufs=2)
                        gather_block(xg_blk, xl_dram, gat_idx, b, gat_ch,
                                     GCOLS, HC)
                        rg_blk = wp.tile([P, K * HC], BF, tag="rg_blk", bufs=1)
                        gather_block(rg_blk, xr_dram, gxr_idx, b, gat_ch,
                                     GCOLS, HC)

                        sel_blk = wp.tile([P, K * P], BF, tag="sel_blk", bufs=2)
                        nc.vector.tensor_tensor(
                            out=sel_blk[:].rearrange("p (k q) -> p k q", k=K),
                            in0=gdstl[:, b * K:(b + 1) * K].unsqueeze(2)
                                .to_broadcast([P, K, P]),
                            in1=iota_bf[:].unsqueeze(1).to_broadcast([P, K, P]),
                            op=mybir.AluOpType.is_equal)

                        z_blk = wp.tile([P, K * HC], BF, tag="z_blk", bufs=1)
                        nc.vector.tensor_add(z_blk[:], xg_blk[:], rg_blk[:])
                        # leaky relu 0.2: 0.2*z on ScalarE, max on DVE
                        z02 = wp.tile([P, K * HC], BF, tag="rhs_blk", bufs=1)
                        nc.scalar.activation(z02[:], z_blk[:],
                                             mybir.ActivationFunctionType.Copy,
                                             scale=0.2)
                        nc.vector.tensor_max(z_blk[:], z_blk[:], z02[:])
                        nc.vector.tensor_mul(
                            z_blk[:].rearrange("p (k d) -> p k d", k=K),
                            z_blk[:].rearrange("p (k d) -> p k d", k=K),
                            att_sb[:].unsqueeze(1).to_broadcast([P, K, HC]))
                        t1 = wp.tile([P, K * HEADS * 32], BF, tag="t1", bufs=1)
                        zv = z_blk[:].rearrange("p (s c) -> p s c", c=HID)
                        nc.vector.tensor_add(
                            t1[:].rearrange("p (s c) -> p s c", c=32),
                            zv[:, :, 0:32], zv[:, :, 32:64])
                        t2 = wp.tile([P, K * HEADS * 16], BF, tag="t2", bufs=1)
                        t1v = t1[:].rearrange("p (s c) -> p s c", c=32)
                        nc.vector.tensor_add(
                            t2[:].rearrange("p (s c) -> p s c", c=16),
                            t1v[:, :, 0:16], t1v[:, :, 16:32])
                        alpha_blk = wp.tile([P, K * HEADS], F32, tag="alpha", bufs=2)
                        nc.vector.reduce_sum(
                            out=alpha_blk[:],
                            in_=t2[:].rearrange("p (k h c) -> p k h c", k=K, c=16),
                            axis=mybir.AxisListType.X)
                        ea_blk = wp.tile([P, K * HEADS], F32, tag="ea", bufs=2)
                        nc.scalar.activation(ea_blk[:], alpha_blk[:],
                                             mybir.ActivationFunctionType.Exp)
                        eam_blk = wp.tile([P, K * HEADS], BF, tag="eam", bufs=2)
                        nc.vector.tensor_mul(
                            eam_blk[:].rearrange("p (k h) -> p k h", k=K),
                            ea_blk[:].rearrange("p (k h) -> p k h", k=K),
                            gmask[:, b * K:(b + 1) * K].unsqueeze(2)
                                .to_broadcast([P, K, HEADS]))
                        rhs_blk = wp.tile([P, K * HC], BF, tag="rhs_blk", bufs=1)
                        nc.vector.tensor_mul(
                            rhs_blk[:].rearrange("p (k h c) -> p k h c", k=K, c=HID),
                            xg_blk[:].rearrange("p (k h c) -> p k h c", k=K, c=HID),
                            eam_blk[:].rearrange("p (k h) -> p k h", k=K)
                                .unsqueeze(3).to_broadcast([P, K, HEADS, HID]))

                        for k in range(K):
                            nc.tensor.matmul(out_ps[:],
                                             lhsT=sel_blk[:, k * P:(k + 1) * P],
                                             rhs=rhs_blk[:, k * HC:(k + 1) * HC],
                                             start=(k == 0), stop=(k == K - 1))
                            nc.tensor.matmul(den_ps[:],
                                             lhsT=sel_blk[:, k * P:(k + 1) * P],
                                             rhs=eam_blk[:, k * HEADS:(k + 1) * HEADS],
                                             start=(k == 0), stop=(k == K - 1))

                        den_sb = wp.tile([P, HEADS], F32, tag="den", bufs=2)
                        nc.vector.tensor_scalar_add(den_sb[:], den_ps[:], 1e-16)
                        rec = wp.tile([P, HEADS], F32, tag="rec", bufs=2)
                        nc.vector.reciprocal(rec[:], den_sb[:])
                        res_sb = wp.tile([P, HC], BF, tag="res", bufs=2)
                        nc.sync.dma_start(res_sb[:], res_dram[b * P:(b + 1) * P, :])
                        o = wp.tile([P, HC], F32, tag="o", bufs=2)
                        nc.vector.tensor_mul(
                            o[:].rearrange("p (h c) -> p h c", c=HID),
                            out_ps[:].rearrange("p (h c) -> p h c", c=HID),
                            rec[:].unsqueeze(2).to_broadcast([P, HEADS, HID]))
                        nc.vector.tensor_add(o[:], o[:], res_sb[:])
                        nc.vector.tensor_add(o[:], o[:], bias_sb[:])
                        hn = wp.tile([P, HC], BF, tag="hn", bufs=2)
                        if layer == 2:
                            # ScalarE Lrelu has a fixed 0.01 slope - exactly
                            # what this layer needs
                            nc.scalar.activation(
                                hn[:], o[:], mybir.ActivationFunctionType.Lrelu)
                        else:
                            neg = wp.tile([P, HC], F32, tag="neg", bufs=2)
                            nc.vector.tensor_scalar_min(neg[:], o[:], 0.0)
                            nc.scalar.activation(neg[:], neg[:],
                                                 mybir.ActivationFunctionType.Exp)
                            nc.vector.tensor_scalar_max(hn[:], o[:], 0.0)
                            nc.vector.tensor_add(hn[:], hn[:], neg[:])
                            nc.vector.tensor_scalar_add(hn[:], hn[:], -1.0)
                        if h3_bounce is not None:
                            nc.sync.dma_start(h3_bounce[b * P:(b + 1) * P, :], hn[:])
                        for ch in range(4):
                            nc.sync.dma_start(
                                hT_next[:, ch * NPAD + b * P: ch * NPAD + (b + 1) * P],
                                hn[:, ch * P:(ch + 1) * P], transpose=True)

            # =========================================================
            # GAT1 (input dim 64): replicate xl GEMM from h1_full
            # =========================================================
            xl1_d = dr.tile([NFULL, HC], BF)
            xr1_d = dr.tile([NPAD, HC], BF)
            res1_d = dr.tile([NPAD, HC], BF)
            hT2 = hTp.tile([P, 4 * NPAD], BF, tag="hT")

            with tc.tile_pool(name="g1_sb", bufs=1) as wp:
                nc.gpsimd.memset(hT2[:], 0.0)
                wl_sb = wp.tile([HID, HC], BF)
                nc.sync.dma_start(wl_sb[:], wls[1][:])
                wr_sb = wp.tile([HID, HC], BF)
                nc.sync.dma_start(wr_sb[:], wrs[1][:])
                wres_sb = wp.tile([HID, HC], BF)
                nc.sync.dma_start(wres_sb[:], wress[1][:])

                with tc.tile_pool(name="g1t_ps", bufs=2, space="PSUM") as pp:
                    h1T = wp.tile([HID, NFULL], BF)
                    for nt in range(NT):
                        h1_tile = wp.tile([P, HID], BF, tag="h1t", bufs=3)
                        nc.sync.dma_start(h1_tile[:],
                                          h1_full_d[nt * P:(nt + 1) * P, :])
                        h1T_ps = pp.tile([HID, P], BF, tag="tr", space="PSUM")
                        nc.tensor.transpose(h1T_ps[:], h1_tile[:], ident_bf[:])
                        nc.vector.tensor_copy(h1T[:, nt * P:(nt + 1) * P], h1T_ps[:])

                    for nt in range(NT):
                        xl_ps = pp.tile([P, HC], F32, tag="xl", space="PSUM")
                        nc.tensor.matmul(xl_ps[:], lhsT=h1T[:, nt * P:(nt + 1) * P],
                                         rhs=wl_sb[:], start=True, stop=True)
                        xl_sb = wp.tile([P, HC], BF, tag="xl_sb", bufs=3)
                        nc.vector.tensor_copy(xl_sb[:], xl_ps[:])
                        nc.sync.dma_start(xl1_d[nt * P:(nt + 1) * P, :], xl_sb[:])
                    h1oT = wp.tile([HID, NPAD], BF)
                    for b in range(BLOCKS):
                        h1o_tile = wp.tile([P, HID], BF, tag="h1ot", bufs=3)
                        nc.sync.dma_start(h1o_tile[:],
                                          h1_bounce[b * P:(b + 1) * P, :])
                        h1oT_ps = pp.tile([HID, P], BF, tag="tr", space="PSUM")
                        nc.tensor.transpose(h1oT_ps[:], h1o_tile[:], ident_bf[:])
                        nc.vector.tensor_copy(h1oT[:, b * P:(b + 1) * P], h1oT_ps[:])
                    for b in range(BLOCKS):
                        xr_ps = pp.tile([P, HC], F32, tag="xr", space="PSUM")
                        nc.tensor.matmul(xr_ps[:], lhsT=h1oT[:, b * P:(b + 1) * P],
                                         rhs=wr_sb[:], start=True, stop=True)
                        xr_sb = wp.tile([P, HC], BF, tag="xr_sb", bufs=3)
                        nc.vector.tensor_copy(xr_sb[:], xr_ps[:])
                        nc.sync.dma_start(xr1_d[b * P:(b + 1) * P, :], xr_sb[:])
                        res_ps = pp.tile([P, HC], F32, tag="xr", space="PSUM")
                        nc.tensor.matmul(res_ps[:], lhsT=h1oT[:, b * P:(b + 1) * P],
                                         rhs=wres_sb[:], start=True, stop=True)
                        res_sb2 = wp.tile([P, HC], BF, tag="res_sb2", bufs=3)
                        nc.vector.tensor_copy(res_sb2[:], res_ps[:])
                        nc.sync.dma_start(res1_d[b * P:(b + 1) * P, :], res_sb2[:])

            gat_edge_phase(1, xl1_d, xr1_d, res1_d, hT2)

            # =========================================================
            # GAT2 / GAT3 (input dim 512, bf16)
            # =========================================================
            def gemm_own(wp, hT_sb, wl_d, wr_d, wres_d, xl_bounce, xr_d, res_d):
                with tc.tile_pool(name="gemm_w", bufs=1) as wpool:
                    wl_sb = wpool.tile([P, 4 * HC], BF, tag="wlw")
                    wr_sb = wpool.tile([P, 4 * HC], BF, tag="wrw")
                    wres_sb = wpool.tile([P, 4 * HC], BF, tag="wresw")
                    for kc in range(4):
                        nc.sync.dma_start(wl_sb[:, kc * HC:(kc + 1) * HC],
                                          wl_d[kc * P:(kc + 1) * P, :])
                        nc.sync.dma_start(wr_sb[:, kc * HC:(kc + 1) * HC],
                                          wr_d[kc * P:(kc + 1) * P, :])
                        nc.sync.dma_start(wres_sb[:, kc * HC:(kc + 1) * HC],
                                          wres_d[kc * P:(kc + 1) * P, :])
                    with tc.tile_pool(name="gemm_ps", bufs=2, space="PSUM") as pp:
                        for b in range(BLOCKS):
                            xl_ps = pp.tile([P, HC], F32, tag="xl", space="PSUM")
                            xr_ps = pp.tile([P, HC], F32, tag="xr", space="PSUM")
                            res_ps = pp.tile([P, HC], F32, tag="resp", space="PSUM")
                            for kc in range(4):
                                lhs = hT_sb[:, kc * NPAD + b * P:
                                            kc * NPAD + (b + 1) * P]
                                nc.tensor.matmul(
                                    xl_ps[:], lhsT=lhs,
                                    rhs=wl_sb[:, kc * HC:(kc + 1) * HC],
                                    start=(kc == 0), stop=(kc == 3))
                                nc.tensor.matmul(
                                    xr_ps[:], lhsT=lhs,
                                    rhs=wr_sb[:, kc * HC:(kc + 1) * HC],
                                    start=(kc == 0), stop=(kc == 3))
                                nc.tensor.matmul(
                                    res_ps[:], lhsT=lhs,
                                    rhs=wres_sb[:, kc * HC:(kc + 1) * HC],
                                    start=(kc == 0), stop=(kc == 3))
                            xl_sb = wp.tile([P, HC], BF, tag="xl_sb", bufs=3)
                            nc.vector.tensor_copy(xl_sb[:], xl_ps[:])
                            nc.sync.dma_start(xl_bounce[b * P:(b + 1) * P, :],
                                              xl_sb[:])
                            xr_sb = wp.tile([P, HC], BF, tag="xr_sb", bufs=3)
                            nc.vector.tensor_copy(xr_sb[:], xr_ps[:])
                            nc.sync.dma_start(xr_d[b * P:(b + 1) * P, :], xr_sb[:])
                            res_sb2 = wp.tile([P, HC], BF, tag="res_sb2", bufs=3)
                            nc.vector.tensor_copy(res_sb2[:], res_ps[:])
                            nc.sync.dma_start(res_d[b * P:(b + 1) * P, :],
                                              res_sb2[:])

            hT3 = hTp.tile([P, 4 * NPAD], BF, tag="hT")
            xl2_b = dr.tile([NPAD, HC], BF)
            xl2_full = dr.tile([NFULL, HC], BF, addr_space="Shared")
            xr2_d = dr.tile([NPAD, HC], BF)
            res2_d = dr.tile([NPAD, HC], BF)
            with tc.tile_pool(name="g2_sb", bufs=1) as wp:
                nc.gpsimd.memset(hT3[:], 0.0)
                gemm_own(wp, hT2, wls[2], wrs[2], wress[2], xl2_b, xr2_d, res2_d)
                nc.gpsimd.collective_compute(
                    "AllGather", mybir.AluOpType.bypass,
                    replica_groups=[list(range(NCORES))],
                    ins=[xl2_b.opt()], outs=[xl2_full.opt()])
                gat_edge_phase(2, xl2_full, xr2_d, res2_d, hT3)

            hT4 = hTp.tile([P, 4 * NPAD], BF, tag="hT")
            xl3_b = dr.tile([NPAD, HC], BF)
            xl3_full = dr.tile([NFULL, HC], BF, addr_space="Shared")
            xr3_d = dr.tile([NPAD, HC], BF)
            res3_d = dr.tile([NPAD, HC], BF)
            h3_bounce = dr.tile([NPAD, HC], BF)
            h3_full = dr.tile([NFULL, HC], BF, addr_space="Shared")
            with tc.tile_pool(name="g3_sb", bufs=1) as wp:
                nc.gpsimd.memset(hT4[:], 0.0)
                gemm_own(wp, hT3, wls[3], wrs[3], wress[3], xl3_b, xr3_d, res3_d)
                nc.gpsimd.collective_compute(
                    "AllGather", mybir.AluOpType.bypass,
                    replica_groups=[list(range(NCORES))],
                    ins=[xl3_b.opt()], outs=[xl3_full.opt()])
                gat_edge_phase(3, xl3_full, xr3_d, res3_d, hT4,
                               h3_bounce=h3_bounce)

            nc.gpsimd.collective_compute(
                "AllGather", mybir.AluOpType.bypass,
                replica_groups=[list(range(NCORES))],
                ins=[h3_bounce.opt()], outs=[h3_full.opt()])

            # =========================================================
            # SAGE + pooling (bf16)
            # =========================================================
            pool_b = dr.tile([G, G], F32)
            pool_full = dr.tile([G, G], F32, addr_space="Shared")
            with tc.tile_pool(name="sg_sb", bufs=1) as wp:
                swn_sb = wp.tile([P, 4 * HID], BF)
                swr_sb = wp.tile([P, 4 * HID], BF)
                for kc in range(4):
                    nc.sync.dma_start(swn_sb[:, kc * HID:(kc + 1) * HID],
                                      swn_in[kc * P:(kc + 1) * P, :])
                    nc.sync.dma_start(swr_sb[:, kc * HID:(kc + 1) * HID],
                                      swr_in[kc * P:(kc + 1) * P, :])
                sbn_sb = wp.tile([P, HID], F32)
                nc.sync.dma_start(sbn_sb[:], sbn_in[:])

                with (
                    tc.tile_pool(name="sg_ps", bufs=1, space="PSUM") as pp,
                    tc.tile_pool(name="pool_ps_pool", bufs=1, space="PSUM") as plp,
                ):
                    pool_ps = plp.tile([G, G], F32, space="PSUM")
                    for b in range(BLOCKS):
                        hg_blk = wp.tile([P, K_sage * HC], BF, tag="hg", bufs=2)
                        gather_block(hg_blk, h3_full, sage_idx, b, sage_ch,
                                     SCOLS, HC)
                        agg_ps = pp.tile([P, HC], F32, tag="agg", bufs=2,
                                         space="PSUM")
                        cnt_ps = pp.tile([P, 1], F32, tag="cnt", bufs=1,
                                         space="PSUM")
                        sels_blk = wp.tile([P, K_sage * P], BF, tag="sels", bufs=2)
                        nc.vector.tensor_tensor(
                            out=sels_blk[:].rearrange("p (k q) -> p k q", k=K_sage),
                            in0=sdstl[:, b * K_sage:(b + 1) * K_sage].unsqueeze(2)
                                .to_broadcast([P, K_sage, P]),
                            in1=iota_f[:].unsqueeze(1).to_broadcast([P, K_sage, P]),
                            op=mybir.AluOpType.is_equal)
                        nc.vector.tensor_mul(
                            sels_blk[:].rearrange("p (k q) -> p k q", k=K_sage),
                            sels_blk[:].rearrange("p (k q) -> p k q", k=K_sage),
                            smask[:, b * K_sage:(b + 1) * K_sage].unsqueeze(2)
                                .to_broadcast([P, K_sage, P]))
                        for k in range(K_sage):
                            nc.tensor.matmul(agg_ps[:],
                                             lhsT=sels_blk[:, k * P:(k + 1) * P],
                                             rhs=hg_blk[:, k * HC:(k + 1) * HC],
                                             start=(k == 0), stop=(k == K_sage - 1))
                            nc.tensor.matmul(cnt_ps[:],
                                             lhsT=sels_blk[:, k * P:(k + 1) * P],
                                             rhs=ones_col[:],
                                             start=(k == 0), stop=(k == K_sage - 1))
                        cnt_sb = wp.tile([P, 1], F32, tag="cnt_sb", bufs=2)
                        nc.vector.tensor_scalar_max(cnt_sb[:], cnt_ps[:], 1.0)
                        recc = wp.tile([P, 1], F32, tag="recc", bufs=2)
                        nc.vector.reciprocal(recc[:], cnt_sb[:])
                        mean = wp.tile([P, HC], BF, tag="mean", bufs=2)
                        nc.vector.tensor_mul(mean[:], agg_ps[:],
                                             recc[:].to_broadcast([P, HC]))
                        sage_ps = pp.tile([P, HID], F32, tag="sage", bufs=1,
                                          space="PSUM")
                        for kc in range(4):
                            mT_sb = wp.tile([P, P], BF, tag="mT_sb", bufs=2)
                            nc.sync.dma_start(mT_sb[:],
                                              mean[:, kc * P:(kc + 1) * P],
                                              transpose=True)
                            nc.tensor.matmul(sage_ps[:], lhsT=mT_sb[:],
                                             rhs=swn_sb[:, kc * HID:(kc + 1) * HID],
                                             start=(kc == 0), stop=False)
                            lhs_h = hT4[:, kc * NPAD + b * P: kc * NPAD + (b + 1) * P]
                            nc.tensor.matmul(sage_ps[:], lhsT=lhs_h,
                                             rhs=swr_sb[:, kc * HID:(kc + 1) * HID],
                                             start=False, stop=(kc == 3))
                        sage_sb = wp.tile([P, HID], BF, tag="sage_sb", bufs=2)
                        nc.vector.tensor_add(sage_sb[:], sage_ps[:], sbn_sb[:])
                        nc.scalar.activation(sage_sb[:], sage_sb[:],
                                             mybir.ActivationFunctionType.Relu)
                        nc.tensor.matmul(pool_ps[:], lhsT=B_sb[:, b * G:(b + 1) * G],
                                         rhs=sage_sb[:], start=(b == 0),
                                         stop=(b == BLOCKS - 1))

                    pool_sb = wp.tile([G, G], F32)
                    nc.vector.tensor_copy(pool_sb[:], pool_ps[:])
                    nc.sync.dma_start(pool_b[:], pool_sb[:])

                nc.gpsimd.collective_compute(
                    "AllReduce", mybir.AluOpType.add,
                    replica_groups=[list(range(NCORES))],
                    ins=[pool_b.opt()], outs=[pool_full.opt()])

                with tc.tile_pool(name="head_ps", bufs=1, space="PSUM") as pp:
                    poolf = wp.tile([G, G], F32)
                    nc.sync.dma_start(poolf[:], pool_full[:])
                    nc.vector.tensor_mul(poolf[:], poolf[:],
                                         rgc[:].to_broadcast([G, G]))
                    pT_ps = pp.tile([G, G], F32, tag="pT", space="PSUM")
                    nc.tensor.transpose(pT_ps[:], poolf[:], ident[:G, :G])
                    pT_sb = wp.tile([G, G], F32)
                    nc.vector.tensor_copy(pT_sb[:], pT_ps[:])
                    ow_sb = wp.tile([HID, 2], F32)
                    nc.sync.dma_start(ow_sb[:], ow_in[:])
                    ob_sb = wp.tile([G, 2], F32)
                    nc.sync.dma_start(ob_sb[:], ob_in[:])
                    lg_ps = pp.tile([G, 2], F32, tag="lg", space="PSUM")
                    nc.tensor.matmul(lg_ps[:], lhsT=pT_sb[:], rhs=ow_sb[:],
                                     start=True, stop=True)
                    lg = wp.tile([G, 2], F32)
                    nc.vector.tensor_add(lg[:], lg_ps[:], ob_sb[:])
                    mx = wp.tile([G, 1], F32)
                    nc.vector.reduce_max(out=mx[:], in_=lg[:],
                                         axis=mybir.AxisListType.X)
                    zm = wp.tile([G, 2], F32)
                    nc.vector.tensor_sub(zm[:], lg[:], mx[:].to_broadcast([G, 2]))
                    ez = wp.tile([G, 2], F32)
                    nc.scalar.activation(ez[:], zm[:],
                                         mybir.ActivationFunctionType.Exp)
                    s = wp.tile([G, 1], F32)
                    nc.vector.reduce_sum(out=s[:], in_=ez[:],
                                         axis=mybir.AxisListType.X)
                    ls = wp.tile([G, 1], F32)
                    nc.scalar.activation(ls[:], s[:],
                                         mybir.ActivationFunctionType.Ln)
                    res_out = wp.tile([G, 2], F32)
                    nc.vector.tensor_sub(res_out[:], zm[:],
                                         ls[:].to_broadcast([G, 2]))
                    nc.sync.dma_start(out[:], res_out[:])

    nc.compile()
    return nc


def _make_in_maps(pre):
    w = pre["weights"]
    in_maps = []
    for c in range(NCORES):
        pc = pre["per_core"][c]
        m = {
            "x_gather": pre["x_gather"],
            "xT_aug": pre["xT_aug"][c],
            "gat_idx16": pc["gat_idx16"], "gxr_idx16": pc["gxr_idx16"],
            "sage_idx16": pc["sage_idx16"],
            "gat_dstl": pc["gdstl"], "gat_mask": pc["gmask"],
            "sage_dstl": pc["sdstl"], "sage_mask": pc["smask"],
            "B_onehot": pre["B_all"][c],
            "recip_gcnt": pre["recip_gcnt"],
            "mlw_aug": w["mlw_aug"], "muw": w["muw"], "mub_rep": w["mub_rep"],
            "sage_wn": w["sage_wn"], "sage_wr": w["sage_wr"], "sbn_rep": w["sbn_rep"],
            "out_w": w["out_w"], "ob_rep": w["ob_rep"],
        }
        for i in (1, 2, 3):
            m[f"wl{i}"] = w[f"wl{i}"]
            m[f"wr{i}"] = w[f"wr{i}"]
            m[f"wres{i}"] = w[f"wres{i}"]
            m[f"att_rep{i}"] = w[f"att_rep{i}"]
            m[f"b_rep{i}"] = w[f"b_rep{i}"]
        in_maps.append(m)
    return in_maps


def kernel(**inputs):
    pre = _preprocess(inputs)
    key = (pre["K_gat"], pre["K_sage"])
    if key not in _CACHE:
        _CACHE[key] = _build(*key)
    nc = _CACHE[key]
    in_maps = _make_in_maps(pre)
    res = bass_utils.run_bass_kernel_spmd(nc, in_maps, core_ids=list(range(NCORES)))
    return res.results[0]["out"]



# revision 6
# speedup vs baseline: 1.8933x; 1.8933x over previous
"""Trainium2 Bass kernel for ExplainableDumplingGNN (MPNN -> 3x GAT -> SAGE -> pool).

Self-contained: takes full inputs, shards node blocks + incident edges across
8 NeuronCores internally, runs one SPMD Bass kernel, returns [64, 2] log-probs.

v2 design vs baseline:
- xr[dst] per-edge broadcast via PE matmul against a host-built one-hot sel2
  (kills the 1KB/edge rg DRAM gather entirely); z^T accumulates in PSUM from
  that matmul plus PE transposes of the gathered xl[src] chunks.
- leaky-relu(0.2) as a single ScalarE Prelu pass PSUM->SBUF; attention logits
  alpha = att . lrelu(z) as 4 small PE matmuls (att block-diagonal rhs); exp on
  ScalarE. DVE only does the xg*ea multiply. All funcs live in one activation
  table (exp_and_others) -> no table thrash.
- Padded edge slots have all-zero sel1/sel2 columns -> no mask tensors/ops.
- dma_gather spread over 4 SWDGE queues; sel1/sel2 SBUF-resident (graph const).
- hT produced by PE transposes (no DMA transpose descriptors storms).
- MPNN folded to 2 matmuls via host-precomputed W1 = mlw_aug @ muw_top,
  W2 = mlw_aug @ muw_bot; relu(lrelu(x,.1)) == relu(x).
- SAGE mean degree reciprocal host-precomputed (graph const).
"""
import itertools
import sys

sys.path.insert(0, "/opt/trn_rl_repo")

import ml_dtypes
import numpy as np

import concourse.bacc as bacc
import concourse.mybir as mybir
import concourse.tile as tile
from concourse import bass_utils
from concourse.masks import make_identity

P = 128
NCORES = 8
N = 10000
NBLK = 1250
NPAD = 1280
BLOCKS = 10
NFULL = NPAD * NCORES  # 10240
D_IN = 8
HID = 64
HEADS = 8
HC = 512
G = 64
XG = 128  # padded x row for MPNN dma_gather (256B rows, bf16)
MAXT = 8  # max 128-edge tiles per dma_gather (1024 descriptors)
NQ = 4  # SWDGE queues

F32 = mybir.dt.float32
BF = mybir.dt.bfloat16
I16 = mybir.dt.int16

BF_NP = ml_dtypes.bfloat16
AF = mybir.ActivationFunctionType

_CACHE = {}


def _chunks(K):
    out = []
    k0 = 0
    while k0 < K:
        n = min(MAXT, K - k0)
        out.append((k0, n))
        k0 += n
    return out


def _pad_id(n):
    return (n // NBLK) * NPAD + (n % NBLK)


def _split_blocks(es_pad, ed_local):
    order = np.argsort(ed_local, kind="stable")
    es_pad, ed_local = es_pad[order], ed_local[order]
    per_block = []
    K = 1
    for b in range(BLOCKS):
        m = (ed_local >= b * P) & (ed_local < (b + 1) * P)
        s, d = es_pad[m], ed_local[m] - b * P
        per_block.append((s, d))
        K = max(K, (len(s) + P - 1) // P)
    return per_block, K


def _pack_idx16(flat):
    """[n] int -> [128, n//16] int16, wrapped in 16 partitions, replicated x8."""
    n = len(flat)
    ncols = n // 16
    a = np.zeros((P, ncols), np.int16)
    j = np.arange(n)
    a[j % 16, j // 16] = flat.astype(np.int16)
    for c in range(1, 8):
        a[16 * c:16 * (c + 1)] = a[:16]
    return a


def _edge_arrays(per_block, K):
    """Returns (src_flat [BLOCKS, K*P], sel1 [P, BLOCKS*K*P], sel2 same).

    sel1[p, (b*K+k)*P + q] = (dst(edge b,k*P+p) == q) and valid
    sel2[p, (b*K+k)*P + j] = (dst(edge b,k*P+j) == p) and valid
    Padded slots get all-zero sel columns (so they contribute nothing).
    """
    T = BLOCKS * K
    src_flat = np.zeros((BLOCKS, K * P), np.int32)
    sel1 = np.zeros((P, T * P), BF_NP)
    sel2 = np.zeros((P, T * P), BF_NP)
    for b, (s, d) in enumerate(per_block):
        n = len(s)
        slots = K * P
        s_pad = np.zeros(slots, np.int32)
        s_pad[:n] = s
        if 0 < n < slots:
            s_pad[n:] = s[n - 1]
        src_flat[b] = s_pad
        j = np.arange(n)
        k, p = j // P, j % P
        # tile t = b*K + k ; edge slot p ; dst q = d[j]
        sel1[p, (b * K + k) * P + d[:n]] = 1.0
        sel2[d[:n], (b * K + k) * P + p] = 1.0
    return src_flat, sel1, sel2


def _pack_block_idx(src_flat, K):
    cols = K * P // 16
    out = np.zeros((P, BLOCKS * cols), np.int16)
    for b in range(BLOCKS):
        out[:, b * cols:(b + 1) * cols] = _pack_idx16(src_flat[b])
    return out


def _preprocess(inputs):
    x = np.asarray(inputs["x"], np.float32)
    ei = np.asarray(inputs["edge_index"], np.int32)
    batch = np.asarray(inputs["batch"], np.int32)
    src, dst = ei[0], ei[1]

    blocks_per_core = []
    K_gat = 1
    K_sage = 1
    for c in range(NCORES):
        lo, hi = c * NBLK, (c + 1) * NBLK
        m = (dst >= lo) & (dst < hi)
        s_c = _pad_id(src[m]).astype(np.int32)
        d_c = (dst[m] - lo).astype(np.int32)
        own = np.arange(lo, hi, dtype=np.int32)
        gs = np.concatenate([s_c, _pad_id(own).astype(np.int32)])
        gd = np.concatenate([d_c, (own - lo)])
        gat_blocks, kg = _split_blocks(gs, gd)
        sage_blocks, ks = _split_blocks(s_c, d_c)
        K_gat = max(K_gat, kg)
        K_sage = max(K_sage, ks)
        blocks_per_core.append((gat_blocks, sage_blocks))

    per_core = []
    for c in range(NCORES):
        gat_blocks, sage_blocks = blocks_per_core[c]
        gsrc_flat, gsel1, gsel2 = _edge_arrays(gat_blocks, K_gat)
        ssrc_flat, ssel1, _ = _edge_arrays(sage_blocks, K_sage)
        # SAGE mean denominators (real edge count per dst, no self loops)
        deg = np.zeros(NPAD, np.float32)
        for b, (s, d) in enumerate(sage_blocks):
            np.add.at(deg, b * P + d, 1.0)
        recip_cnt = (1.0 / np.maximum(deg, 1.0)).reshape(BLOCKS, P).T.copy()
        per_core.append(dict(
            gat_idx16=_pack_block_idx(gsrc_flat, K_gat),
            sage_idx16=_pack_block_idx(ssrc_flat, K_sage),
            gsel1=gsel1, gsel2=gsel2, ssel1=ssel1,
            recip_cnt=recip_cnt.astype(np.float32),
        ))

    B_all = []
    for c in range(NCORES):
        Bm = np.zeros((P, BLOCKS * G), np.float32)
        loc = np.arange(NBLK)
        gids = batch[c * NBLK:(c + 1) * NBLK]
        Bm[loc % P, (loc // P) * G + gids] = 1.0
        B_all.append(Bm.astype(BF_NP))

    gcnt = np.bincount(batch, minlength=G).astype(np.float32)
    recip_gcnt = (1.0 / np.maximum(gcnt, 1.0)).reshape(G, 1).astype(np.float32)

    # x rows for MPNN gather: [x, 1, 0...] bf16, 256B rows
    x_gather = np.zeros((NFULL, XG), BF_NP)
    for c in range(NCORES):
        x_gather[c * NPAD:c * NPAD + NBLK, :D_IN] = x[c * NBLK:(c + 1) * NBLK]
        x_gather[c * NPAD:c * NPAD + NBLK, D_IN] = 1.0
    xT_aug = []
    for c in range(NCORES):
        t = np.zeros((D_IN + 1, NPAD), np.float32)
        t[:D_IN, :NBLK] = x[c * NBLK:(c + 1) * NBLK].T
        t[D_IN, :] = 1.0
        xT_aug.append(t.astype(BF_NP))

    w = {}
    mlw_aug = np.concatenate(
        [np.asarray(inputs["mpnn_lin_w"], np.float32),
         np.asarray(inputs["mpnn_lin_b"], np.float32)[None, :]], axis=0)
    muw = np.asarray(inputs["mpnn_upd_w"], np.float32)
    w["W1"] = (mlw_aug @ muw[:HID]).astype(BF_NP)   # [9, 64]
    w["W2"] = (mlw_aug @ muw[HID:]).astype(BF_NP)   # [9, 64]
    w["mub_rep"] = np.tile(np.asarray(inputs["mpnn_upd_b"], np.float32)[None, :],
                           (P, 1))
    for i in (1, 2, 3):
        w[f"wl{i}"] = np.asarray(inputs[f"g{i}_wl"], np.float32).astype(BF_NP)
        w[f"wr{i}"] = np.asarray(inputs[f"g{i}_wr"], np.float32).astype(BF_NP)
        w[f"wres{i}"] = np.asarray(inputs[f"g{i}_res"], np.float32).astype(BF_NP)
        att = np.asarray(inputs[f"g{i}_att"], np.float32)  # [HEADS, HID]
        attT = np.zeros((P, 4 * HEADS), np.float32)
        for j in range(4):
            for pp in range(P):
                ch = j * P + pp
                h = ch // HID
                attT[pp, j * HEADS + h] = att[h, ch % HID]
        w[f"attT{i}"] = attT.astype(BF_NP)
        w[f"b_rep{i}"] = np.tile(
            np.asarray(inputs[f"g{i}_b"], np.float32)[None, :], (P, 1))
    w["sage_wn"] = np.asarray(inputs["sage_wn"], np.float32).astype(BF_NP)
    w["sage_wr"] = np.asarray(inputs["sage_wr"], np.float32).astype(BF_NP)
    w["sbn_rep"] = np.tile(np.asarray(inputs["sage_bn"], np.float32)[None, :],
                           (P, 1))
    w["out_w"] = np.asarray(inputs["out_w"], np.float32)
    w["ob_rep"] = np.tile(np.asarray(inputs["out_b"], np.float32)[None, :],
                          (G, 1))

    return dict(
        K_gat=K_gat, K_sage=K_sage, per_core=per_core,
        B_all=B_all, recip_gcnt=recip_gcnt,
        x_gather=x_gather, xT_aug=xT_aug, weights=w,
    )


def _build(K_gat, K_sage):
    nc = bacc.Bacc("TRN2", target_bir_lowering=False, debug=False,
                   num_devices=NCORES, num_swdge_queues=NQ)

    TG = BLOCKS * K_gat
    TS = BLOCKS * K_sage
    GCOLS = K_gat * P // 16
    SCOLS = K_sage * P // 16

    x_gather_in = nc.dram_tensor("x_gather", [NFULL, XG], BF, kind="ExternalInput")
    xT_aug_in = nc.dram_tensor("xT_aug", [D_IN + 1, NPAD], BF, kind="ExternalInput")
    gat_idx_in = nc.dram_tensor("gat_idx16", [P, BLOCKS * GCOLS], I16,
                                kind="ExternalInput")
    sage_idx_in = nc.dram_tensor("sage_idx16", [P, BLOCKS * SCOLS], I16,
                                 kind="ExternalInput")
    gsel1_in = nc.dram_tensor("gsel1", [P, TG * P], BF, kind="ExternalInput")
    gsel2_in = nc.dram_tensor("gsel2", [P, TG * P], BF, kind="ExternalInput")
    ssel1_in = nc.dram_tensor("ssel1", [P, TS * P], BF, kind="ExternalInput")
    rc_in = nc.dram_tensor("recip_cnt", [P, BLOCKS], F32, kind="ExternalInput")
    B_in = nc.dram_tensor("B_onehot", [P, BLOCKS * G], BF, kind="ExternalInput")
    rgc_in = nc.dram_tensor("recip_gcnt", [G, 1], F32, kind="ExternalInput")

    W1_in = nc.dram_tensor("W1", [D_IN + 1, HID], BF, kind="ExternalInput")
    W2_in = nc.dram_tensor("W2", [D_IN + 1, HID], BF, kind="ExternalInput")
    mub_in = nc.dram_tensor("mub_rep", [P, HID], F32, kind="ExternalInput")
    wls, wrs, wress, atts, brs = {}, {}, {}, {}, {}
    for i in (1, 2, 3):
        ind = HID if i == 1 else HC
        wls[i] = nc.dram_tensor(f"wl{i}", [ind, HC], BF, kind="ExternalInput")
        wrs[i] = nc.dram_tensor(f"wr{i}", [ind, HC], BF, kind="ExternalInput")
        wress[i] = nc.dram_tensor(f"wres{i}", [ind, HC], BF, kind="ExternalInput")
        atts[i] = nc.dram_tensor(f"attT{i}", [P, 4 * HEADS], BF,
                                 kind="ExternalInput")
        brs[i] = nc.dram_tensor(f"b_rep{i}", [P, HC], F32, kind="ExternalInput")
    swn_in = nc.dram_tensor("sage_wn", [HC, HID], BF, kind="ExternalInput")
    swr_in = nc.dram_tensor("sage_wr", [HC, HID], BF, kind="ExternalInput")
    sbn_in = nc.dram_tensor("sbn_rep", [P, HID], F32, kind="ExternalInput")
    ow_in = nc.dram_tensor("out_w", [HID, 2], F32, kind="ExternalInput")
    ob_in = nc.dram_tensor("ob_rep", [G, 2], F32, kind="ExternalInput")

    out = nc.dram_tensor("out", [G, 2], F32, kind="ExternalOutput")

    gat_ch = _chunks(K_gat)
    sage_ch = _chunks(K_sage)
    qcyc = itertools.cycle(range(NQ))

    with tile.TileContext(nc) as tc:
        with (
            tc.tile_pool(name="const", bufs=1) as cp,
            tc.tile_pool(name="hTp", bufs=2) as hTp,
            tc.tile_pool(name="dram", bufs=1, space="DRAM") as dr,
        ):
            ident = cp.tile([P, P], F32)
            make_identity(nc, ident[:])
            ident_bf = cp.tile([P, P], BF)
            nc.vector.tensor_copy(ident_bf[:], ident[:])

            gat_idx = cp.tile([P, BLOCKS * GCOLS], I16)
            nc.sync.dma_start(gat_idx[:], gat_idx_in[:])
            sage_idx = cp.tile([P, BLOCKS * SCOLS], I16)
            nc.sync.dma_start(sage_idx[:], sage_idx_in[:])
            B_sb = cp.tile([P, BLOCKS * G], BF)
            nc.sync.dma_start(B_sb[:], B_in[:])
            rgc = cp.tile([G, 1], F32)
            nc.sync.dma_start(rgc[:], rgc_in[:])
            rc_sb = cp.tile([P, BLOCKS], F32)
            nc.sync.dma_start(rc_sb[:], rc_in[:])
            h1T_own = cp.tile([HID, NPAD], BF)

            def gather_block(dst_blk, src_dram, idx_sb, b, k0, nt, cols, width):
                nidx = nt * P
                nc.gpsimd.dma_gather(
                    dst_blk[:, 0:nt * width].rearrange("p (k d) -> p k d", k=nt),
                    src_dram[:],
                    idx_sb[:, b * cols + k0 * P // 16:
                           b * cols + (k0 + nt) * P // 16],
                    nidx, nidx, width, queue_num=next(qcyc))

            # =========================================================
            # Stage 0: MPNN -> h1 (own nodes), h1T_own
            # =========================================================
            with (
                tc.tile_pool(name="mp_sb", bufs=1) as wp,
                tc.tile_pool(name="mp_ps", bufs=1, space="PSUM") as pp,
            ):
                ssel1_sb = wp.tile([P, TS * P], BF)
                nc.sync.dma_start(ssel1_sb[:], ssel1_in[:])
                xT_sb = wp.tile([D_IN + 1, NPAD], BF)
                nc.sync.dma_start(xT_sb[:], xT_aug_in[:])
                W1_sb = wp.tile([D_IN + 1, HID], BF)
                nc.sync.dma_start(W1_sb[:], W1_in[:])
                W2_sb = wp.tile([D_IN + 1, HID], BF)
                nc.sync.dma_start(W2_sb[:], W2_in[:])
                mub_sb = wp.tile([P, HID], F32)
                nc.sync.dma_start(mub_sb[:], mub_in[:])

                for b in range(BLOCKS):
                    xs_ps = pp.tile([P, XG], F32, tag="xs", bufs=2, space="PSUM")
                    nck = 0
                    for (k0, nt) in sage_ch:
                        xgm = wp.tile([P, MAXT * XG], BF, tag="xgm", bufs=3)
                        gather_block(xgm, x_gather_in, sage_idx, b, k0, nt,
                                     SCOLS, XG)
                        for k in range(nt):
                            nc.tensor.matmul(
                                xs_ps[:],
                                lhsT=ssel1_sb[:, (b * K_sage + k0 + k) * P:
                                              (b * K_sage + k0 + k + 1) * P],
                                rhs=xgm[:, k * XG:(k + 1) * XG],
                                start=(nck == 0), stop=(nck == K_sage - 1))
                            nck += 1
                    xs_sb = wp.tile([P, XG], BF, tag="xs_sb", bufs=2)
                    nc.vector.tensor_copy(xs_sb[:], xs_ps[:])
                    xsT_ps = pp.tile([P, P], F32, tag="xsT", bufs=2, space="PSUM")
                    nc.tensor.matmul(xsT_ps[:], lhsT=xs_sb[:], rhs=ident_bf[:],
                                     start=True, stop=True)
                    xsT_sb = wp.tile([P, P], BF, tag="xsT_sb", bufs=2)
                    nc.vector.tensor_copy(xsT_sb[:], xsT_ps[:])
                    h1_ps = pp.tile([P, HID], F32, tag="h1", bufs=2, space="PSUM")
                    nc.tensor.matmul(h1_ps[:], lhsT=xT_sb[:, b * P:(b + 1) * P],
                                     rhs=W1_sb[:], start=True, stop=False)
                    nc.tensor.matmul(h1_ps[:], lhsT=xsT_sb[0:D_IN + 1, :],
                                     rhs=W2_sb[:], start=False, stop=True)
                    h1f = wp.tile([P, HID], F32, tag="h1f", bufs=2)
                    nc.vector.tensor_add(h1f[:], h1_ps[:], mub_sb[:])
                    h1_sb = wp.tile([P, HID], BF, tag="h1_sb", bufs=2)
                    nc.scalar.activation(h1_sb[:], h1f[:], AF.Relu)
                    h1T_ps = pp.tile([HID, P], F32, tag="h1T", bufs=2,
                                     space="PSUM")
                    nc.tensor.matmul(h1T_ps[:], lhsT=h1_sb[:], rhs=ident_bf[:],
                                     start=True, stop=True)
                    nc.vector.tensor_copy(h1T_own[:, b * P:(b + 1) * P],
                                          h1T_ps[:])

            # =========================================================
            # GAT layers
            # =========================================================
            def gemm_phase(layer, hT_sb, xl_b, xr_own, res_own):
                """xl (own) -> DRAM xl_b; xr_own/res_own SBUF; bias folded."""
                ind = HID if layer == 1 else HC
                nchunk = ind // P if layer != 1 else 1
                with (
                    tc.tile_pool(name=f"gw{layer}", bufs=1) as wpool,
                    tc.tile_pool(name=f"gw{layer}_ps", bufs=2,
                                 space="PSUM") as pp,
                ):
                    if layer == 1:
                        wl_sb = wpool.tile([HID, HC], BF, tag="wl")
                        nc.sync.dma_start(wl_sb[:], wls[1][:])
                        wr_sb = wpool.tile([HID, HC], BF, tag="wr")
                        nc.sync.dma_start(wr_sb[:], wrs[1][:])
                        wres_sb = wpool.tile([HID, HC], BF, tag="wres")
                        nc.sync.dma_start(wres_sb[:], wress[1][:])
                    else:
                        wl_sb = wpool.tile([P, 4 * HC], BF, tag="wl")
                        wr_sb = wpool.tile([P, 4 * HC], BF, tag="wr")
                        wres_sb = wpool.tile([P, 4 * HC], BF, tag="wres")
                        for kc in range(4):
                            nc.sync.dma_start(wl_sb[:, kc * HC:(kc + 1) * HC],
                                              wls[layer][kc * P:(kc + 1) * P, :])
                            nc.sync.dma_start(wr_sb[:, kc * HC:(kc + 1) * HC],
                                              wrs[layer][kc * P:(kc + 1) * P, :])
                            nc.sync.dma_start(wres_sb[:, kc * HC:(kc + 1) * HC],
                                              wress[layer][kc * P:(kc + 1) * P, :])
                    brep_sb = wpool.tile([P, HC], F32, tag="brep")
                    nc.sync.dma_start(brep_sb[:], brs[layer][:])

                    def lhs(b, kc):
                        if layer == 1:
                            return h1T_own[:, b * P:(b + 1) * P]
                        return hT_sb[:, kc * NPAD + b * P: kc * NPAD + (b + 1) * P]

                    def wsl(w_sb, kc):
                        if layer == 1:
                            return w_sb[:]
                        return w_sb[:, kc * HC:(kc + 1) * HC]

                    # xl first so the AllGather can start ASAP
                    for b in range(BLOCKS):
                        xl_ps = pp.tile([P, HC], F32, tag="xl", space="PSUM")
                        for kc in range(nchunk):
                            nc.tensor.matmul(xl_ps[:], lhsT=lhs(b, kc),
                                             rhs=wsl(wl_sb, kc),
                                             start=(kc == 0),
                                             stop=(kc == nchunk - 1))
                        xl_sb = wpool.tile([P, HC], BF, tag="xl_sb", bufs=3)
                        nc.vector.tensor_copy(xl_sb[:], xl_ps[:])
                        nc.sync.dma_start(xl_b[b * P:(b + 1) * P, :], xl_sb[:])
                    for b in range(BLOCKS):
                        xr_ps = pp.tile([P, HC], F32, tag="xr", space="PSUM")
                        res_ps = pp.tile([P, HC], F32, tag="res", space="PSUM")
                        for kc in range(nchunk):
                            nc.tensor.matmul(xr_ps[:], lhsT=lhs(b, kc),
                                             rhs=wsl(wr_sb, kc),
                                             start=(kc == 0),
                                             stop=(kc == nchunk - 1))
                            nc.tensor.matmul(res_ps[:], lhsT=lhs(b, kc),
                                             rhs=wsl(wres_sb, kc),
                                             start=(kc == 0),
                                             stop=(kc == nchunk - 1))
                        nc.vector.tensor_copy(xr_own[:, b * HC:(b + 1) * HC],
                                              xr_ps[:])
                        nc.vector.tensor_add(res_own[:, b * HC:(b + 1) * HC],
                                             res_ps[:], brep_sb[:])

            def edge_phase(layer, xl_full, xr_own, res_own, hT_next,
                           h3_bounce=None):
                K = K_gat
                with (
                    tc.tile_pool(name=f"e{layer}", bufs=1) as wp,
                    tc.tile_pool(name=f"e{layer}_ps", bufs=1, space="PSUM") as pp,
                ):
                    att_sb = wp.tile([P, 4 * HEADS], BF, tag="att")
                    nc.sync.dma_start(att_sb[:], atts[layer][:])
                    for b in range(BLOCKS):
                        out_ps = pp.tile([P, HC], F32, tag="out", bufs=2,
                                         space="PSUM")
                        den_ps = pp.tile([P, HEADS], F32, tag="den", bufs=1,
                                         space="PSUM")
                        nck = 0
                        for (k0, nt) in gat_ch:
                            xg = wp.tile([P, MAXT * HC], BF, tag="xg", bufs=3)
                            gather_block(xg, xl_full, gat_idx, b, k0, nt,
                                         GCOLS, HC)
                            for k in range(nt):
                                t = b * K + k0 + k
                                s1 = sel1[:, t * P:(t + 1) * P]
                                s2 = sel2[:, t * P:(t + 1) * P]
                                xgt = xg[:, k * HC:(k + 1) * HC]
                                zT_ps = pp.tile([P, HC], F32, tag="zT", bufs=2,
                                                space="PSUM")
                                for j in range(4):
                                    nc.tensor.matmul(
                                        zT_ps[:, j * P:(j + 1) * P],
                                        lhsT=xr_own[:, b * HC + j * P:
                                                    b * HC + (j + 1) * P],
                                        rhs=s2, start=True, stop=False)
                                    nc.tensor.matmul(
                                        zT_ps[:, j * P:(j + 1) * P],
                                        lhsT=xgt[:, j * P:(j + 1) * P],
                                        rhs=ident_bf[:], start=False, stop=True)
                                zlr = wp.tile([P, HC], BF, tag="zlr", bufs=2)
                                nc.scalar.activation(zlr[:], zT_ps[:], AF.Prelu,
                                                     alpha=0.2)
                                al_ps = pp.tile([P, HEADS], F32, tag="al",
                                                bufs=2, space="PSUM")
                                for j in range(4):
                                    nc.tensor.matmul(
                                        al_ps[:],
                                        lhsT=zlr[:, j * P:(j + 1) * P],
                                        rhs=att_sb[:, j * HEADS:(j + 1) * HEADS],
                                        start=(j == 0), stop=(j == 3))
                                ea = wp.tile([P, HEADS], BF, tag="ea", bufs=2)
                                nc.scalar.activation(ea[:], al_ps[:], AF.Exp)
                                rhs_sb = wp.tile([P, HC], BF, tag="rhs", bufs=2)
                                nc.vector.tensor_mul(
                                    rhs_sb[:].rearrange("p (h c) -> p h c",
                                                        c=HID),
                                    xgt.rearrange("p (h c) -> p h c", c=HID),
                                    ea[:].unsqueeze(2)
                                        .to_broadcast([P, HEADS, HID]))
                                nc.tensor.matmul(den_ps[:], lhsT=s1, rhs=ea[:],
                                                 start=(nck == 0),
                                                 stop=(nck == K - 1))
                                nc.tensor.matmul(out_ps[:], lhsT=s1,
                                                 rhs=rhs_sb[:],
                                                 start=(nck == 0),
                                                 stop=(nck == K - 1))
                                nck += 1

                        den_sb = wp.tile([P, HEADS], F32, tag="dsb", bufs=2)
                        nc.vector.tensor_scalar_add(den_sb[:], den_ps[:], 1e-16)
                        rec = wp.tile([P, HEADS], F32, tag="rec", bufs=2)
                        nc.vector.reciprocal(rec[:], den_sb[:])
                        o = wp.tile([P, HC], F32, tag="o", bufs=2)
                        nc.vector.tensor_mul(
                            o[:].rearrange("p (h c) -> p h c", c=HID),
                            out_ps[:].rearrange("p (h c) -> p h c", c=HID),
                            rec[:].unsqueeze(2).to_broadcast([P, HEADS, HID]))
                        nc.vector.tensor_add(o[:], o[:],
                                             res_own[:, b * HC:(b + 1) * HC])
                        hn = wp.tile([P, HC], BF, tag="hn", bufs=2)
                        if layer == 2:
                            nc.scalar.activation(hn[:], o[:], AF.Prelu,
                                                 alpha=0.01)
                        else:
                            neg = wp.tile([P, HC], BF, tag="neg", bufs=2)
                            nc.vector.tensor_scalar_min(neg[:], o[:], 0.0)
                            expn = wp.tile([P, HC], F32, tag="expn", bufs=2)
                            nc.scalar.activation(expn[:], neg[:], AF.Exp)
                            nc.vector.tensor_scalar_max(hn[:], o[:], 0.0)
                            nc.vector.tensor_add(hn[:], hn[:], expn[:])
                            nc.vector.tensor_scalar_add(hn[:], hn[:], -1.0)
                        if h3_bounce is not None:
                            nc.sync.dma_start(h3_bounce[b * P:(b + 1) * P, :],
                                              hn[:])
                        hnT_ps = pp.tile([P, HC], F32, tag="hnT", bufs=1,
                                         space="PSUM")
                        for j in range(4):
                            nc.tensor.matmul(hnT_ps[:, j * P:(j + 1) * P],
                                             lhsT=hn[:, j * P:(j + 1) * P],
                                             rhs=ident_bf[:], start=True,
                                             stop=True)
                        nc.vector.tensor_copy(
                            hT_next[:].rearrange("p (c n) -> p c n", n=NPAD)
                            [:, :, b * P:(b + 1) * P],
                            hnT_ps[:].rearrange("p (c n) -> p c n", n=P))

            hT2 = hTp.tile([P, 4 * NPAD], BF, tag="hT")
            hT3 = hTp.tile([P, 4 * NPAD], BF, tag="hT")
            hT4 = hTp.tile([P, 4 * NPAD], BF, tag="hT")

            h3_bounce = dr.tile([NPAD, HC], BF)
            h3_full = dr.tile([NFULL, HC], BF, addr_space="Shared")
            with tc.tile_pool(name="gatp", bufs=1) as gp:
                sel1 = gp.tile([P, TG * P], BF)
                nc.sync.dma_start(sel1[:], gsel1_in[:])
                sel2 = gp.tile([P, TG * P], BF)
                nc.sync.dma_start(sel2[:], gsel2_in[:])
                xr_own = gp.tile([P, BLOCKS * HC], BF)
                res_own = gp.tile([P, BLOCKS * HC], BF)
                for i, (hT_in, hT_out) in enumerate(
                        [(None, hT2), (hT2, hT3), (hT3, hT4)], start=1):
                    xl_b = dr.tile([NPAD, HC], BF, tag=f"xlb{i}")
                    xl_full = dr.tile([NFULL, HC], BF, addr_space="Shared",
                                      tag=f"xlf{i}")
                    gemm_phase(i, hT_in, xl_b, xr_own, res_own)
                    nc.gpsimd.collective_compute(
                        "AllGather", mybir.AluOpType.bypass,
                        replica_groups=[list(range(NCORES))],
                        ins=[xl_b.opt()], outs=[xl_full.opt()])
                    edge_phase(i, xl_full, xr_own, res_own, hT_out,
                               h3_bounce=h3_bounce if i == 3 else None)

            nc.gpsimd.collective_compute(
                "AllGather", mybir.AluOpType.bypass,
                replica_groups=[list(range(NCORES))],
                ins=[h3_bounce.opt()], outs=[h3_full.opt()])

            # =========================================================
            # SAGE + pooling
            # =========================================================
            pool_b = dr.tile([G, HID], F32)
            pool_full = dr.tile([G, HID], F32, addr_space="Shared")
            with (
                tc.tile_pool(name="sg_sb", bufs=1) as wp,
                tc.tile_pool(name="sg_ps", bufs=1, space="PSUM") as pp,
                tc.tile_pool(name="pool_ps_pool", bufs=1, space="PSUM") as plp,
            ):
                ssel1_sb = wp.tile([P, TS * P], BF)
                nc.sync.dma_start(ssel1_sb[:], ssel1_in[:])
                swn_sb = wp.tile([P, 4 * HID], BF)
                swr_sb = wp.tile([P, 4 * HID], BF)
                for kc in range(4):
                    nc.sync.dma_start(swn_sb[:, kc * HID:(kc + 1) * HID],
                                      swn_in[kc * P:(kc + 1) * P, :])
                    nc.sync.dma_start(swr_sb[:, kc * HID:(kc + 1) * HID],
                                      swr_in[kc * P:(kc + 1) * P, :])
                sbn_sb = wp.tile([P, HID], F32)
                nc.sync.dma_start(sbn_sb[:], sbn_in[:])

                pool_ps = plp.tile([G, HID], F32, space="PSUM")
                for b in range(BLOCKS):
                    agg_ps = pp.tile([P, HC], F32, tag="agg", bufs=2,
                                     space="PSUM")
                    nck = 0
                    for (k0, nt) in sage_ch:
                        hg = wp.tile([P, MAXT * HC], BF, tag="hg", bufs=3)
                        gather_block(hg, h3_full, sage_idx, b, k0, nt,
                                     SCOLS, HC)
                        for k in range(nt):
                            nc.tensor.matmul(
                                agg_ps[:],
                                lhsT=ssel1_sb[:, (b * K_sage + k0 + k) * P:
                                              (b * K_sage + k0 + k + 1) * P],
                                rhs=hg[:, k * HC:(k + 1) * HC],
                                start=(nck == 0), stop=(nck == K_sage - 1))
                            nck += 1
                    mean = wp.tile([P, HC], BF, tag="mean", bufs=2)
                    nc.vector.tensor_mul(mean[:], agg_ps[:],
                                         rc_sb[:, b:b + 1].to_broadcast([P, HC]))
                    mT_ps = pp.tile([P, HC], F32, tag="mT", bufs=1, space="PSUM")
                    for j in range(4):
                        nc.tensor.matmul(mT_ps[:, j * P:(j + 1) * P],
                                         lhsT=mean[:, j * P:(j + 1) * P],
                                         rhs=ident_bf[:], start=True, stop=True)
                    mT_sb = wp.tile([P, HC], BF, tag="mT_sb", bufs=2)
                    nc.vector.tensor_copy(mT_sb[:], mT_ps[:])
                    sage_ps = pp.tile([P, HID], F32, tag="sage", bufs=1,
                                      space="PSUM")
                    for kc in range(4):
                        nc.tensor.matmul(sage_ps[:],
                                         lhsT=mT_sb[:, kc * P:(kc + 1) * P],
                                         rhs=swn_sb[:, kc * HID:(kc + 1) * HID],
                                         start=(kc == 0), stop=False)
                        nc.tensor.matmul(
                            sage_ps[:],
                            lhsT=hT4[:, kc * NPAD + b * P: kc * NPAD + (b + 1) * P],
                            rhs=swr_sb[:, kc * HID:(kc + 1) * HID],
                            start=False, stop=(kc == 3))
                    sgf = wp.tile([P, HID], F32, tag="sgf", bufs=2)
                    nc.vector.tensor_add(sgf[:], sage_ps[:], sbn_sb[:])
                    sage_sb = wp.tile([P, HID], BF, tag="sage_sb", bufs=2)
                    nc.scalar.activation(sage_sb[:], sgf[:], AF.Relu)
                    nc.tensor.matmul(pool_ps[:], lhsT=B_sb[:, b * G:(b + 1) * G],
                                     rhs=sage_sb[:], start=(b == 0),
                                     stop=(b == BLOCKS - 1))

                pool_sb = wp.tile([G, HID], F32)
                nc.vector.tensor_copy(pool_sb[:], pool_ps[:])
                nc.sync.dma_start(pool_b[:], pool_sb[:])

                nc.gpsimd.collective_compute(
                    "AllReduce", mybir.AluOpType.add,
                    replica_groups=[list(range(NCORES))],
                    ins=[pool_b.opt()], outs=[pool_full.opt()])

                with tc.tile_pool(name="head_ps", bufs=1, space="PSUM") as hp:
                    poolf = wp.tile([G, HID], F32)
                    nc.sync.dma_start(poolf[:], pool_full[:])
                    nc.vector.tensor_mul(poolf[:], poolf[:],
                                         rgc[:].to_broadcast([G, HID]))
                    pT_ps = hp.tile([HID, G], F32, tag="pT", space="PSUM")
                    nc.tensor.transpose(pT_ps[:], poolf[:], ident[:G, :G])
                    pT_sb = wp.tile([HID, G], F32)
                    nc.vector.tensor_copy(pT_sb[:], pT_ps[:])
                    ow_sb = wp.tile([HID, 2], F32)
                    nc.sync.dma_start(ow_sb[:], ow_in[:])
                    ob_sb = wp.tile([G, 2], F32)
                    nc.sync.dma_start(ob_sb[:], ob_in[:])
                    lg_ps = hp.tile([G, 2], F32, tag="lg", space="PSUM")
                    nc.tensor.matmul(lg_ps[:], lhsT=pT_sb[:], rhs=ow_sb[:],
                                     start=True, stop=True)
                    lg = wp.tile([G, 2], F32)
                    nc.vector.tensor_add(lg[:], lg_ps[:], ob_sb[:])
                    mx = wp.tile([G, 1], F32)
                    nc.vector.reduce_max(out=mx[:], in_=lg[:],
                                         axis=mybir.AxisListType.X)
                    zm = wp.tile([G, 2], F32)
                    nc.vector.tensor_sub(zm[:], lg[:], mx[:].to_broadcast([G, 2]))
                    ez = wp.tile([G, 2], F32)
                    nc.scalar.activation(ez[:], zm[:], AF.Exp)
                    s = wp.tile([G, 1], F32)
                    nc.vector.reduce_sum(out=s[:], in_=ez[:],
                                         axis=mybir.AxisListType.X)
                    ls = wp.tile([G, 1], F32)
                    nc.scalar.activation(ls[:], s[:], AF.Ln)
                    res_out = wp.tile([G, 2], F32)
                    nc.vector.tensor_sub(res_out[:], zm[:],
                                         ls[:].to_broadcast([G, 2]))
                    nc.sync.dma_start(out[:], res_out[:])

    nc.compile()
    return nc


def _make_in_maps(pre):
    w = pre["weights"]
    in_maps = []
    for c in range(NCORES):
        pc = pre["per_core"][c]
        m = {
            "x_gather": pre["x_gather"],
            "xT_aug": pre["xT_aug"][c],
            "gat_idx16": pc["gat_idx16"], "sage_idx16": pc["sage_idx16"],
            "gsel1": pc["gsel1"], "gsel2": pc["gsel2"], "ssel1": pc["ssel1"],
            "recip_cnt": pc["recip_cnt"],
            "B_onehot": pre["B_all"][c],
            "recip_gcnt": pre["recip_gcnt"],
            "W1": w["W1"], "W2": w["W2"], "mub_rep": w["mub_rep"],
            "sage_wn": w["sage_wn"], "sage_wr": w["sage_wr"],
            "sbn_rep": w["sbn_rep"],
            "out_w": w["out_w"], "ob_rep": w["ob_rep"],
        }
        for i in (1, 2, 3):
            m[f"wl{i}"] = w[f"wl{i}"]
            m[f"wr{i}"] = w[f"wr{i}"]
            m[f"wres{i}"] = w[f"wres{i}"]
            m[f"attT{i}"] = w[f"attT{i}"]
            m[f"b_rep{i}"] = w[f"b_rep{i}"]
        in_maps.append(m)
    return in_maps


def kernel(**inputs):
    pre = _preprocess(inputs)
    key = (pre["K_gat"], pre["K_sage"])
    if key not in _CACHE:
        _CACHE[key] = _build(*key)
    nc = _CACHE[key]
    in_maps = _make_in_maps(pre)
    res = bass_utils.run_bass_kernel_spmd(nc, in_maps, core_ids=list(range(NCORES)))
    return res.results[0]["out"]
